# revision 57
# baseline (speedup 1.0000x reference)
"""Trainium2 Bass kernel for nn_DendroMatrixNN (B=4096, F=256, L0=128, L1=16, E=2048, N=2048).

Algorithm
---------
The reference computes per-sample effective weights
    w1[b] = root1 + einsum('lfe,e->fl', delta1, paths[:, b])
    h[b]  = relu(x[b] @ w1[b]);  out[b] = h[b] @ w2[b]
with paths = path_mat[:, node_idx].  Since node_idx takes at most N=2048
distinct values (and typically only ~1770 are hit), we precompute per-NODE
tables with one big matmul over the unique hit nodes
    A1'[node, (l,f)] = sum_e path_mat[e, node] * delta1[l, f, e] + root1[f, l]
    A2'[node, (l,m)] = sum_e path_mat[e, node] * delta2[m, l, e] + root2[l, m]
then per sample gather row node_idx[b] (indirect DMA) and contract with x[b]
on the vector engine.  Cores shard the L0 dimension (16 l-values per core);
each core produces a partial output (contraction over its l-slice) and the
host sums the 8 partials.

To overlap the per-sample stage with the big matmul, the unique nodes are
split into Q parts; samples are sorted by node (host side) so each part's
samples can be processed as soon as that part's table is written (the delta
weights are re-streamed from HBM per part, which is hidden under the matmul).

The big matmul runs in bf16 (cast during DMA), accumulated in fp32 PSUM.
Root weight rows are broadcast to all partitions via a K=1 ones-matmul and
added during PSUM eviction.
"""

import ml_dtypes
import numpy as np

BF16 = ml_dtypes.bfloat16

# Problem dims (hardcoded per spec nn_DendroMatrixNN_19301583028815)
B, F, L0, L1 = 4096, 256, 128, 16
E, N = 2048, 2048
NCORES = 8
import os as _os
Q_SPLIT = int(_os.environ.get("KERNEL_Q_SPLIT", "5"))
# how many of the 16 per-l dots ride the scalar engine (DVE/ACT balance)
ACT_LS = int(_os.environ.get("KERNEL_ACT_LS", "0"))


def make_cfg(b=B, f=F, l0=L0, l1=L1, e=E, n=N, ncores=NCORES):
    lc = l0 // ncores          # l-values per core
    m1 = lc * f                # A1 columns per core
    m2 = lc * l1               # A2 columns per core
    m = m1 + m2
    assert l0 % ncores == 0 and b % 128 == 0 and e % 128 == 0 and n % 128 == 0
    assert m1 % 512 == 0 and m2 <= 512
    return dict(
        b=b, f=f, l0=l0, l1=l1, e=e, n=n, ncores=ncores,
        lc=lc, m1=m1, m2=m2, m=m,
        ec=e // 128, tc=b // 128, nb1=m1 // 512,
    )


def build_program(cfg, nk_parts, t_parts):
    """Build the single-core SPMD Bass program (same program on all cores,
    per-core data differs).  nk_parts[q] = node-chunks in part q; t_parts[q] =
    sample tiles assigned to part q.  Returns the compiled Bacc object."""
    from contextlib import ExitStack

    import concourse.bass as bass
    import concourse.tile as tile
    from concourse import bacc, mybir

    f32 = mybir.dt.float32
    bf = mybir.dt.bfloat16
    i32 = mybir.dt.int32
    Alu = mybir.AluOpType
    Act = mybir.ActivationFunctionType
    Axis = mybir.AxisListType

    f, l1 = cfg["f"], cfg["l1"]
    m, m1, m2 = cfg["m"], cfg["m1"], cfg["m2"]
    lc, ec, nb1 = cfg["lc"], cfg["ec"], cfg["nb1"]
    nkc = sum(nk_parts)
    t_tot = sum(t_parts)
    nq = len(nk_parts)

    nc = bacc.Bacc("TRN2", target_bir_lowering=False, debug=False)

    # ---- I/O (weights host-prepared in bf16 so loads are plain HWDGE) -------
    # path_h[p, nk, c, j] = path_sel[c*128+p, nk*128+j]  (path_sel = hit-node cols)
    path_h = nc.dram_tensor("path_h", [128, nkc, ec, 128], bf, kind="ExternalInput")
    # delta_main[p, bk, c, j] = deltaT[c*128+p, bk*512+j]   (deltaT = [E, m] per-core)
    dmain = nc.dram_tensor("delta_main", [128, nb1, ec, 512], bf, kind="ExternalInput")
    # delta_tail[p, c, j] = deltaT[c*128+p, m1+j]
    dtail = nc.dram_tensor("delta_tail", [128, ec, m2], bf, kind="ExternalInput")
    # x_t[p, t, f'] = x[sample(t*128+p), f']   (samples sorted by node part)
    x_t = nc.dram_tensor("x_t", [128, t_tot, f], bf, kind="ExternalInput")
    # idx_t[p, t] = local node-row of sample (t*128+p) within its part's table
    idx_t = nc.dram_tensor("idx_t", [128, t_tot], i32, kind="ExternalInput")
    # root_row[0, :] = per-core root weights, layout matching table columns
    root_row = nc.dram_tensor("root_row", [1, m], bf, kind="ExternalInput")
    # partial output, outp[p, t*l1+j] = out_partial[sample(t*128+p), j]
    outp = nc.dram_tensor("outp", [128, t_tot * l1], f32, kind="ExternalOutput")
    # per-node tables (internal scratch in HBM), one per part
    a1t = [
        nc.dram_tensor(f"a1t{q}", [nk_parts[q] * 128, m], bf) for q in range(nq)
    ]

    banks = [(i * 512, 512) for i in range(nb1)] + [(m1, m2)]

    with tile.TileContext(nc) as tc, ExitStack() as ctx:
        pconst = ctx.enter_context(tc.tile_pool(name="const", bufs=1))
        ppath = ctx.enter_context(tc.tile_pool(name="path", bufs=1))
        pdelta = ctx.enter_context(tc.tile_pool(name="delta", bufs=4))
        pevict = ctx.enter_context(tc.tile_pool(name="evict", bufs=4))
        ppsum = ctx.enter_context(tc.tile_pool(name="psum", bufs=8, space="PSUM"))
        pgather = ctx.enter_context(tc.tile_pool(name="gather", bufs=3))
        psmall = ctx.enter_context(tc.tile_pool(name="small", bufs=2))

        # ---- root rows broadcast to all partitions via K=1 ones-matmul ------
        root_sb = pconst.tile([1, m], bf, tag="root")
        nc.sync.dma_start(out=root_sb[:], in_=root_row[:])
        ones_sb = pconst.tile([1, 128], bf, tag="ones")
        nc.vector.memset(ones_sb[:], 1.0)
        rrep = pconst.tile([128, m], bf, tag="rootrep")
        for bi, (col0, bw) in enumerate(banks):
            psr = ppsum.tile([128, bw], f32, tag="psum_mm", name=f"psr{bi}")
            nc.tensor.matmul(
                psr[:], lhsT=ones_sb[:], rhs=root_sb[:, col0:col0 + bw],
                start=True, stop=True,
            )
            nc.vector.tensor_copy(rrep[:, col0:col0 + bw], psr[:])

        # resident path tile; chunks are loaded just-in-time per part below
        # (SWDGE queue; keeps HWDGE free for evict writes)
        path_sb = ppath.tile([128, nkc, ec, 128], bf, tag="path")

        x_sb = pconst.tile([128, t_tot, f], bf, tag="x")
        idx_sb = pconst.tile([128, t_tot], i32, tag="idx")
        out_sb = pconst.tile([128, t_tot * l1], f32, tag="outsb")

        def load_delta(q, bi):
            bw = banks[bi][1]
            dsb = pdelta.tile([128, ec, bw], bf, tag="delta", name=f"dsb{q}_{bi}")
            src = dmain[:, bi, :, :] if bi < nb1 else dtail[:, :, :]
            nc.gpsimd.dma_start(out=dsb[:], in_=src)
            return dsb

        def stage_a(q, nk_base, preloaded):
            nkq = nk_parts[q]
            for bi, (col0, bw) in enumerate(banks):
                dsb = preloaded[bi] if bi in preloaded else load_delta(q, bi)
                for nkl in range(nkq):
                    nk = nk_base + nkl
                    ps = ppsum.tile([128, bw], f32, tag="psum_mm", name=f"ps{q}_{bi}_{nkl}")
                    for c in range(ec):
                        nc.tensor.matmul(
                            ps[:],
                            lhsT=path_sb[:, nk, c, :],
                            rhs=dsb[:, c, :],
                            start=(c == 0), stop=(c == ec - 1),
                        )
                    ev = pevict.tile([128, bw], bf, tag="evict", name=f"ev{q}_{bi}_{nkl}")
                    # eviction adds the broadcast root row (PSUM read -> DVE).
                    # High priority so evicts outrank earlier-emitted stage-B
                    # DVE work: a delayed evict stalls the matmul stream on
                    # the PSUM slot handoff.
                    with tc.high_priority():
                        nc.vector.tensor_tensor(
                            out=ev[:], in0=ps[:], in1=rrep[:, col0:col0 + bw],
                            op=Alu.add,
                        )
                    nc.sync.dma_start(
                        out=a1t[q][nkl * 128:(nkl + 1) * 128, col0:col0 + bw],
                        in_=ev[:],
                    )

        def stage_b(q, t_base):
            for tl in range(t_parts[q]):
                t = t_base + tl
                G = pgather.tile([128, m], bf, tag="G", name=f"G{t}")
                nc.gpsimd.indirect_dma_start(
                    out=G[:],
                    out_offset=None,
                    in_=a1t[q][:, :],
                    in_offset=bass.IndirectOffsetOnAxis(ap=idx_sb[:, t:t + 1], axis=0),
                )
                hpre = psmall.tile([128, lc], f32, tag="hpre", name=f"hpre{t}")
                dummy = psmall.tile([128, f], bf, tag="dummy", name=f"dummy{t}")
                for l in range(ACT_LS, lc):
                    # fused multiply+free-dim-sum on DVE (1x, but one pass)
                    nc.vector.scalar_tensor_tensor(
                        out=dummy[:],
                        in0=G[:, l * f:(l + 1) * f],
                        scalar=1.0,
                        in1=x_sb[:, t, :],
                        op0=Alu.bypass,
                        op1=Alu.mult,
                        accum_out=hpre[:, l:l + 1],
                    )
                for l in range(ACT_LS):
                    # a few l's ride the scalar engine instead: DVE does only
                    # the multiply, ACT's activation-accumulator does the sum
                    prodl = psmall.tile([128, f], bf, tag="prodl", bufs=3,
                                        name=f"prodl{t}_{l}")
                    nc.vector.tensor_tensor(
                        out=prodl[:],
                        in0=G[:, l * f:(l + 1) * f],
                        in1=x_sb[:, t, :],
                        op=Alu.mult,
                    )
                    nc.scalar.activation(
                        out=dummy[:], in_=prodl[:], func=Act.Identity,
                        accum_out=hpre[:, l:l + 1],
                    )
                h = psmall.tile([128, lc], f32, tag="h", name=f"h{t}")
                nc.scalar.activation(out=h[:], in_=hpre[:], func=Act.Relu)
                # layer 2: G2 columns are (m, l) l-minor; prod[p, m', l] then
                # reduce over the innermost l axis
                prod = psmall.tile([128, l1, lc], f32, tag="prod", name=f"prod{t}")
                nc.vector.tensor_tensor(
                    out=prod[:],
                    in0=G[:, m1:m].rearrange("p (m2_ l_) -> p m2_ l_", l_=lc),
                    in1=h[:, None, :].to_broadcast([128, l1, lc]),
                    op=Alu.mult,
                )
                o = out_sb[:, t * l1:(t + 1) * l1]
                nc.vector.tensor_reduce(out=o, in_=prod[:], axis=Axis.X, op=Alu.add)

        def load_path(q, nk_base, skip_first=False):
            for nkl in range(1 if skip_first else 0, nk_parts[q]):
                nk = nk_base + nkl
                nc.gpsimd.dma_start(
                    out=path_sb[:, nk, :, :], in_=path_h[:, nk, :, :]
                )

        nk_base = t_base = 0
        for q in range(nq):
            pre = {}
            if q == 0:
                # the first matmul group needs only path chunk 0 and delta
                # bank 0 -- queue those two ahead of the remaining path chunks
                # so the PE starts ~5 us earlier
                nc.gpsimd.dma_start(
                    out=path_sb[:, 0, :, :], in_=path_h[:, 0, :, :]
                )
                pre = {0: load_delta(0, 0)}
                load_path(q, nk_base, skip_first=True)
            else:
                load_path(q, nk_base)
            stage_a(q, nk_base, pre)
            if q == 0:
                # stage-B-only inputs: load after part 0's matmul stream is
                # queued so they don't delay the first matmuls; SWDGE queue so
                # they don't delay evict writes on the HWDGE FIFO either
                nc.gpsimd.dma_start(out=x_sb[:], in_=x_t[:])
                nc.gpsimd.dma_start(out=idx_sb[:], in_=idx_t[:])
            stage_b(q, t_base)
            if t_parts[q]:
                # flush this part's output rows so the final drain is tiny
                osl = slice(t_base * l1, (t_base + t_parts[q]) * l1)
                nc.sync.dma_start(out=outp[:, osl], in_=out_sb[:, osl])
            nk_base += nk_parts[q]
            t_base += t_parts[q]

    nc.compile()
    return nc


def host_prep(cfg, nq, x, node_idx, path_mat, root_lin1, root_lin2, delta_mat1,
              delta_mat2):
    """Dedup nodes, sort samples by node part, shard/relayout per core.

    Returns (in_maps, nk_parts, t_parts, sample_order) where sample_order[k]
    is the original sample index at slot k (-1 for padding)."""
    f, l1, e = cfg["f"], cfg["l1"], cfg["e"]
    lc, m1, m2 = cfg["lc"], cfg["m1"], cfg["m2"]
    ec, nb1 = cfg["ec"], cfg["nb1"]
    b = cfg["b"]

    x = np.asarray(x, np.float32)
    node_idx = np.asarray(node_idx, np.int32)
    path_mat = np.asarray(path_mat, np.float32)
    root_lin1 = np.asarray(root_lin1, np.float32)
    root_lin2 = np.asarray(root_lin2, np.float32)
    delta_mat1 = np.asarray(delta_mat1, np.float32)
    delta_mat2 = np.asarray(delta_mat2, np.float32)

    # dedup hit nodes; order them by sample multiplicity (descending) so the
    # last part -- whose per-sample stage is the serial tail of the kernel --
    # holds the fewest samples, and busy nodes get the most overlap time
    uniq, inv, counts = np.unique(node_idx, return_inverse=True,
                                  return_counts=True)
    k = len(uniq)
    ordr = np.argsort(-counts, kind="stable")
    rank = np.empty_like(ordr)
    rank[ordr] = np.arange(k)
    pos = rank[inv]                     # per-sample node position, count-sorted
    nkc = -(-k // 128)
    n_eff = nkc * 128
    path_sel = np.zeros((e, n_eff), np.float32)
    path_sel[:, :k] = path_mat[:, uniq[ordr]]

    # split node-chunks into nq roughly equal parts (each big enough that its
    # matmuls stay compute-bound over the re-streamed delta weights)
    nq = min(nq, nkc)
    base, rem = divmod(nkc, nq)
    nk_parts = [base + 1] * rem + [base] * (nq - rem)

    # assign samples to parts, sorted, padded to whole 128-sample tiles
    bounds = np.cumsum([0] + nk_parts) * 128
    t_parts = []
    order = []      # original sample index per slot, -1 = pad
    local_idx = []  # local table row per slot
    for q in range(nq):
        sel = np.where((pos >= bounds[q]) & (pos < bounds[q + 1]))[0]
        tq = -(-len(sel) // 128) if len(sel) else 0
        pad = tq * 128 - len(sel)
        order.extend(sel.tolist() + [-1] * pad)
        local_idx.extend((pos[sel] - bounds[q]).tolist() + [0] * pad)
        t_parts.append(tq)
    t_tot = sum(t_parts)
    order = np.asarray(order, np.int64)
    local_idx = np.asarray(local_idx, np.int32)

    x_sorted = np.zeros((t_tot * 128, f), np.float32)
    valid = order >= 0
    x_sorted[valid] = x[order[valid]]
    x_t = np.ascontiguousarray(
        x_sorted.reshape(t_tot, 128, f).transpose(1, 0, 2), BF16
    )
    idx_t = np.ascontiguousarray(local_idx.reshape(t_tot, 128).T)
    path_h = np.ascontiguousarray(
        # [e, n_eff] -> [p, nk, c, j]
        path_sel.reshape(ec, 128, nkc, 128).transpose(1, 2, 0, 3), BF16
    )

    in_maps = []
    for c in range(cfg["ncores"]):
        lsl = slice(c * lc, (c + 1) * lc)
        d1t = delta_mat1[lsl].reshape(lc * f, e).T          # [e, m1], cols (l, f)
        # A2 columns are (m, l) l-minor so stage B can reduce over l innermost
        d2t = delta_mat2[:, lsl, :].transpose(2, 0, 1).reshape(e, lc * l1)  # [e, m2]
        dm = d1t.reshape(ec, 128, m1).transpose(1, 0, 2)    # [p, c, m1]
        delta_main = np.ascontiguousarray(
            dm.reshape(128, ec, nb1, 512).transpose(0, 2, 1, 3), BF16
        )
        delta_tail = np.ascontiguousarray(
            d2t.reshape(ec, 128, m2).transpose(1, 0, 2), BF16
        )
        root_row = np.concatenate(
            [root_lin1[:, lsl].T.reshape(-1), root_lin2[lsl, :].T.reshape(-1)]
        ).astype(BF16)[None, :]
        in_maps.append({
            "path_h": path_h,
            "delta_main": delta_main,
            "delta_tail": delta_tail,
            "x_t": x_t,
            "idx_t": idx_t,
            "root_row": np.ascontiguousarray(root_row),
        })
    return in_maps, tuple(nk_parts), tuple(t_parts), order


def host_finish(cfg, per_core_outs, t_parts, order):
    """Sum per-core partial outputs and un-sort back to [B, L1]."""
    b, l1 = cfg["b"], cfg["l1"]
    t_tot = sum(t_parts)
    tot = np.zeros((128, t_tot * l1), np.float32)
    for o in per_core_outs:
        tot += o.reshape(128, t_tot * l1)
    # slot (t*128 + p) holds sample order[t*128+p]
    slots = tot.reshape(128, t_tot, l1).transpose(1, 0, 2).reshape(t_tot * 128, l1)
    out = np.zeros((b, l1), np.float32)
    valid = order >= 0
    out[order[valid]] = slots[valid]
    return out


_PROG_CACHE = {}


def _get_program(cfg, nk_parts, t_parts):
    key = (tuple(sorted(cfg.items())), nk_parts, t_parts)
    if key not in _PROG_CACHE:
        _PROG_CACHE[key] = build_program(cfg, nk_parts, t_parts)
    return _PROG_CACHE[key]


def run(trace=False, **inputs):
    """Run on hardware; returns (output, BassKernelResults)."""
    from concourse.bass_utils import run_bass_kernel_spmd

    cfg = make_cfg()
    in_maps, nk_parts, t_parts, order = host_prep(cfg, Q_SPLIT, **inputs)
    nc = _get_program(cfg, nk_parts, t_parts)
    res = run_bass_kernel_spmd(
        nc, in_maps, list(range(cfg["ncores"])), trace=trace
    )
    out = host_finish(cfg, [r["outp"] for r in res.results], t_parts, order)
    return out, res


def kernel(**inputs) -> np.ndarray:
    out, _ = run(trace=False, **inputs)
    return out
